# revision 1
# baseline (speedup 1.0000x reference)
"""MultiHeadDiffAttention Trainium2 kernel.

Strategy (8 NeuronCores, SPMD):
  - Shard: batch (B=2) x head-groups (16 heads -> 4 groups of 4).
    Core c handles b = c//4, heads 4*(c%4) .. 4*(c%4)+3.
  - Differential attention is folded into a single 128-dim attention per head:
      q' = [q1 * scale | q2 * (-lam*scale)],  k' = [k1 | k2]
    so logits = scale*(q1k1 - lam*q2k2) come from ONE 128-contraction matmul.
  - Logits are computed transposed (A^T[s,t]) so that exp(A^T) tiles feed the
    O^T = V^T P^T matmul directly (contraction over s on partitions), with the
    softmax denominator Z[t] obtained by a ones-column matmul over the same
    exp tiles.  No max-subtraction is needed (logits are O(1) for this data).
  - Per-core output is the head-group's slice of out @ W_proj (row-parallel);
    the host sums the 4 partials per batch element.

All matmuls run as float32r (full-speed fp32 on the PE; free dim >= 256
keeps fp32r at the bf16 rate, so full fp32 data costs nothing here).
exp() runs on ScalarE over paired 1024-wide PSUM tiles to amortize the
per-instruction access latency (ACT is otherwise the attention bottleneck).
"""

import math

import numpy as np

B, T, E = 2, 2048, 2048
N_HEAD = 16
HD = 64                       # per-component head dim (q1/k1/q2/k2)
DV = 128                      # v head dim
SCALE = HD ** -0.5
LAMBDA_INIT = 0.8 - 0.6 * math.exp(-0.3 * (1 - 1))
P = 128
NHC = 4                       # heads per core
CQ = NHC * DV                 # 512: per-core q'/k'/v width
N_CORES = 8
NE = E // P                   # 16 contraction chunks
NS = T // P                   # 16 s chunks

_NC_CACHE = None


def _build_nc():
    import concourse.mybir as mybir
    import concourse.tile as tile
    from concourse import bacc

    f32 = mybir.dt.float32
    f32r = mybir.dt.float32r
    bf16 = mybir.dt.bfloat16
    EXP = mybir.ActivationFunctionType.Exp

    nc = bacc.Bacc("TRN2", target_bir_lowering=False, debug=False,
                   num_devices=N_CORES)
    xT = nc.dram_tensor("xT", [E, T], f32r, kind="ExternalInput").ap()
    wq = nc.dram_tensor("wq", [E, CQ], f32r, kind="ExternalInput").ap()
    wk = nc.dram_tensor("wk", [E, CQ], f32r, kind="ExternalInput").ap()
    wv = nc.dram_tensor("wv", [E, CQ], f32r, kind="ExternalInput").ap()
    wp = nc.dram_tensor("wp", [CQ, E], f32r, kind="ExternalInput").ap()
    out = nc.dram_tensor("out", [T, E], f32, kind="ExternalOutput").ap()

    with tile.TileContext(nc) as tc:
        with tc.tile_pool(name="res", bufs=1) as res:
            qt = res.tile([P, NHC, T], f32r, name="qt")     # Q'^T [d, h, t]
            kt = res.tile([P, NHC, T], f32r, name="kt")     # K'^T [d, h, s]
            vsb = res.tile([P, NS, CQ], f32r, name="vsb")   # V [t%128, tc, dv]
            ones_f = res.tile([P, 1], f32, name="ones_f")
            nc.vector.memset(ones_f, 1.0)
            ones_bf = res.tile([P, 1], f32r, name="ones_bf")
            nc.vector.tensor_copy(ones_bf, ones_f)

            # ---------- Phase A: QKV projections ----------
            # Two t-1024 blocks; per block three PSUM rounds (Q, K, V) of
            # 8 banks each, contracting over e with streamed W e-chunks.
            with (
                tc.tile_pool(name="pa_x", bufs=1) as pa_x,
                tc.tile_pool(name="pa_w", bufs=1) as pa_w,
                tc.tile_pool(name="pa_ps", bufs=1, space="PSUM") as pa_ps,
            ):
                for bo in range(2):
                    t0 = bo * 1024
                    xe = [None] * NE

                    # Round Q then K: psum[c*2+half] = [c128, t512]
                    for wsrc, dst in ((wq, qt), (wk, kt)):
                        pss = [
                            pa_ps.tile([P, 512], f32, name="psqk",
                                       tag="pa_ps", bufs=8)
                            for _ in range(8)
                        ]
                        for e in range(NE):
                            if xe[e] is None:
                                # just-in-time x load: paces with the e-loop
                                # instead of a blocking up-front burst
                                xe[e] = pa_x.tile([P, 1024], f32r,
                                                  name=f"xe{e}",
                                                  tag=f"xe{e}", bufs=1)
                                nc.sync.dma_start(
                                    xe[e],
                                    xT[e * P:(e + 1) * P, t0:t0 + 1024])
                            we = pa_w.tile([P, CQ], f32r, name="we",
                                           tag="we", bufs=3)
                            nc.sync.dma_start(we, wsrc[e * P:(e + 1) * P, :])
                            for c in range(4):
                                for half in range(2):
                                    nc.tensor.matmul(
                                        pss[c * 2 + half],
                                        lhsT=we[:, c * P:(c + 1) * P],
                                        rhs=xe[e][:, half * 512:(half + 1) * 512],
                                        start=(e == 0), stop=(e == NE - 1),
                                    )
                        for c in range(4):
                            for half in range(2):
                                nc.vector.tensor_copy(
                                    dst[:, c, t0 + half * 512:t0 + (half + 1) * 512],
                                    pss[c * 2 + half])

                    # Round V: psum[tj] = [t128, dv512]
                    psv = [
                        pa_ps.tile([P, 512], f32, name="psv",
                                   tag="pa_ps", bufs=8)
                        for _ in range(8)
                    ]
                    for e in range(NE):
                        we = pa_w.tile([P, CQ], f32r, name="we",
                                       tag="we", bufs=3)
                        nc.sync.dma_start(we, wv[e * P:(e + 1) * P, :])
                        for tj in range(8):
                            nc.tensor.matmul(
                                psv[tj],
                                lhsT=xe[e][:, tj * P:(tj + 1) * P],
                                rhs=we,
                                start=(e == 0), stop=(e == NE - 1),
                            )
                    for tj in range(8):
                        nc.vector.tensor_copy(vsb[:, bo * 8 + tj, :], psv[tj])

            # ---------- Phase B: attention (+ wpt prefetch) ----------
            with tc.tile_pool(name="pbd", bufs=1) as pbd:
                # prefetch the projection weights during attention
                wpt = pbd.tile([P, NHC, E], f32r, name="wpt")
                nc.sync.dma_start(
                    wpt, wp.rearrange("(ho p) o -> p ho o", p=P))
                ot = pbd.tile([P, NHC, T], f32r, name="ot")  # O^T [dv, h, t]

                with (
                    tc.tile_pool(name="pb_e", bufs=1) as pb_e,
                    tc.tile_pool(name="pb_m", bufs=1) as pb_m,
                    tc.tile_pool(name="pb_ps", bufs=1, space="PSUM") as pb_ps,
                ):
                    for ti2 in range(2):
                        t0 = ti2 * 1024
                        for h in range(NHC):
                            pso = [
                                pb_ps.tile([P, 512], f32, name=f"pso{i}",
                                           tag=f"pso{i}", bufs=1)
                                for i in range(2)
                            ]
                            psz = [
                                pb_ps.tile([1, 512], f32, name=f"psz{i}",
                                           tag=f"psz{i}", bufs=1)
                                for i in range(2)
                            ]
                            for s in range(NS):
                                for half in range(2):
                                    psa = pb_ps.tile([P, 512], f32,
                                                     name="psa",
                                                     tag="psa", bufs=4)
                                    nc.tensor.matmul(
                                        psa,
                                        lhsT=kt[:, h, s * P:(s + 1) * P],
                                        rhs=qt[:, h, t0 + half * 512:
                                               t0 + (half + 1) * 512],
                                        start=True, stop=True,
                                    )
                                    et = pb_e.tile([P, 512], f32r, name="et",
                                                   tag="et", bufs=6)
                                    nc.scalar.activation(et, psa, EXP)
                                    nc.tensor.matmul(
                                        pso[half],
                                        lhsT=vsb[:, s, h * P:(h + 1) * P],
                                        rhs=et,
                                        start=(s == 0), stop=(s == NS - 1),
                                    )
                                    nc.tensor.matmul(
                                        psz[half],
                                        lhsT=ones_bf[:, 0:1],
                                        rhs=et,
                                        start=(s == 0), stop=(s == NS - 1),
                                    )
                            for half in range(2):
                                zr = pb_m.tile([1, 512], f32, name="zr",
                                               tag="zr", bufs=2)
                                nc.vector.reciprocal(zr, psz[half])
                                rb = pb_m.tile([P, 512], f32, name="rb",
                                               tag="rb", bufs=2)
                                nc.gpsimd.partition_broadcast(rb, zr)
                                nc.vector.tensor_mul(
                                    ot[:, h,
                                       t0 + half * 512:t0 + (half + 1) * 512],
                                    pso[half], rb)

                # ---------- Phase D: output projection ----------
                with (
                    tc.tile_pool(name="pd", bufs=1) as pd,
                    tc.tile_pool(name="pd_ps", bufs=1, space="PSUM") as pd_ps,
                ):
                    for tj in range(NS):
                        for eo in range(4):
                            ps = pd_ps.tile([P, 512], f32, name="psd",
                                            tag="psd", bufs=3)
                            for h in range(NHC):
                                nc.tensor.matmul(
                                    ps,
                                    lhsT=ot[:, h, tj * P:(tj + 1) * P],
                                    rhs=wpt[:, h, eo * 512:(eo + 1) * 512],
                                    start=(h == 0), stop=(h == NHC - 1),
                                )
                            osb = pd.tile([P, 512], f32, name="osb",
                                          tag="osb", bufs=3)
                            nc.vector.tensor_copy(osb, ps)
                            nc.sync.dma_start(
                                out[tj * P:(tj + 1) * P,
                                    eo * 512:(eo + 1) * 512],
                                osb)

    nc.compile()
    return nc


def _get_nc():
    global _NC_CACHE
    if _NC_CACHE is None:
        _NC_CACHE = _build_nc()
    return _NC_CACHE


def _shard_inputs(x, W_attn, W_proj, lambda_q1, lambda_k1,
                  lambda_q2, lambda_k2):
    x = np.asarray(x, np.float32)
    W_attn = np.asarray(W_attn, np.float32)
    W_proj = np.asarray(W_proj, np.float32)
    lam = float(np.exp(np.dot(np.asarray(lambda_q1, np.float32),
                              np.asarray(lambda_k1, np.float32)))
                - np.exp(np.dot(np.asarray(lambda_q2, np.float32),
                                np.asarray(lambda_k2, np.float32)))
                + LAMBDA_INIT)
    Cb = E // 2  # 1024: q1/k1/q2/k2 block width in W_attn
    in_maps = []
    for c in range(N_CORES):
        b, hg = divmod(c, 4)
        heads = [4 * hg + j for j in range(NHC)]
        wq_c = np.empty((E, CQ), np.float32)
        wk_c = np.empty((E, CQ), np.float32)
        wv_c = np.empty((E, CQ), np.float32)
        wp_c = np.empty((CQ, E), np.float32)
        for j, h in enumerate(heads):
            wq_c[:, j * P:j * P + HD] = W_attn[:, h * HD:(h + 1) * HD] * SCALE
            wq_c[:, j * P + HD:(j + 1) * P] = (
                W_attn[:, 2 * Cb + h * HD:2 * Cb + (h + 1) * HD]
                * (-lam * SCALE))
            wk_c[:, j * P:j * P + HD] = W_attn[:, Cb + h * HD:Cb + (h + 1) * HD]
            wk_c[:, j * P + HD:(j + 1) * P] = (
                W_attn[:, 3 * Cb + h * HD:3 * Cb + (h + 1) * HD])
            wv_c[:, j * P:(j + 1) * P] = (
                W_attn[:, 4 * Cb + h * DV:4 * Cb + (h + 1) * DV])
            wp_c[j * P:(j + 1) * P, :] = (
                W_proj[h * DV:(h + 1) * DV, :] * (1.0 - LAMBDA_INIT))
        in_maps.append({
            "xT": np.ascontiguousarray(x[b].T),
            "wq": wq_c, "wk": wk_c, "wv": wv_c, "wp": wp_c,
        })
    return in_maps


def _run(inputs, trace=False):
    from concourse.bass_utils import run_bass_kernel_spmd
    nc = _get_nc()
    in_maps = _shard_inputs(**inputs)
    res = run_bass_kernel_spmd(nc, in_maps, list(range(N_CORES)),
                               trace=trace)
    out = np.zeros((B, T, E), np.float32)
    for c in range(N_CORES):
        out[c // 4] += res.results[c]["out"]
    return out, res


def kernel(x, W_attn, W_proj, lambda_q1, lambda_k1, lambda_q2, lambda_k2):
    out, _ = _run(dict(x=x, W_attn=W_attn, W_proj=W_proj,
                       lambda_q1=lambda_q1, lambda_k1=lambda_k1,
                       lambda_q2=lambda_q2, lambda_k2=lambda_k2))
    return out



# revision 2
# speedup vs baseline: 1.0416x; 1.0416x over previous
"""MultiHeadDiffAttention Trainium2 kernel.

Strategy (8 NeuronCores, SPMD):
  - Shard: batch (B=2) x head-groups (16 heads -> 4 groups of 4).
    Core c handles b = c//4, heads 4*(c%4) .. 4*(c%4)+3.
  - Differential attention is folded into a single 128-dim attention per head:
      q' = [q1 * scale | q2 * (-lam*scale)],  k' = [k1 | k2]
    so logits = scale*(q1k1 - lam*q2k2) come from ONE 128-contraction matmul.
  - Logits are computed transposed (A^T[s,t]) so that exp(A^T) tiles feed the
    O^T = V^T P^T matmul directly (contraction over s on partitions).
  - Softmax denominator Z[t]: instead of a ones-column matmul per exp tile
    (which costs the PE a third 512-cycle pass per score tile, ~55us/core),
    the exp tiles (bf16) are pre-reduced over s on the otherwise-idle DVE
    (running sum), and ONE tiny ones-matmul per 512-col unit finishes the
    partition reduction.  PE attention work drops from 3 to 2 matmuls/tile.
  - Since the attention stretch is then ACT(exp)-bound, PE gaps are filled by
    interleaving independent matmul work in program order: the tb1 Q
    projection runs inside attention-tb0, and the tb0 output projection runs
    inside attention-tb1 (the Tile list scheduler slots them into exp waits).
  - k/q/v/exp/wp tensors are bf16 (PE rate is identical; DVE gets 2x; rel-err
    budget 2e-2 comfortably absorbs it); x and W_attn stay f32r.
  - Per-core output is the head-group's slice of out @ W_proj (row-parallel);
    the host sums the 4 partials per batch element.
"""

import math

import numpy as np

B, T, E = 2, 2048, 2048
N_HEAD = 16
HD = 64                       # per-component head dim (q1/k1/q2/k2)
DV = 128                      # v head dim
SCALE = HD ** -0.5
LAMBDA_INIT = 0.8 - 0.6 * math.exp(-0.3 * (1 - 1))
P = 128
NHC = 4                       # heads per core
CQ = NHC * DV                 # 512: per-core q'/k'/v width
N_CORES = 8
NE = E // P                   # 16 contraction chunks
NS = T // P                   # 16 s chunks
NTQ = T // 512                # 4 t units of 512 for attention

_NC_CACHE = None


def _build_nc():
    import concourse.mybir as mybir
    import concourse.tile as tile
    from concourse import bacc

    f32 = mybir.dt.float32
    f32r = mybir.dt.float32r
    bf16 = mybir.dt.bfloat16
    EXP = mybir.ActivationFunctionType.Exp

    nc = bacc.Bacc("TRN2", target_bir_lowering=False, debug=False,
                   num_devices=N_CORES)
    xT = nc.dram_tensor("xT", [E, T], f32r, kind="ExternalInput").ap()
    wq = nc.dram_tensor("wq", [E, CQ], f32r, kind="ExternalInput").ap()
    wk = nc.dram_tensor("wk", [E, CQ], f32r, kind="ExternalInput").ap()
    wv = nc.dram_tensor("wv", [E, CQ], f32r, kind="ExternalInput").ap()
    wp = nc.dram_tensor("wp", [CQ, E], bf16, kind="ExternalInput").ap()
    out = nc.dram_tensor("out", [T, E], f32, kind="ExternalOutput").ap()

    with tile.TileContext(nc) as tc:
        with tc.tile_pool(name="res", bufs=1) as res:
            qt = res.tile([P, NHC, T], bf16, name="qt")     # Q'^T [d, h, t]
            kt = res.tile([P, NHC, T], bf16, name="kt")     # K'^T [d, h, s]
            vsb = res.tile([P, NS, CQ], bf16, name="vsb")   # V [t%128, tc, dv]
            wpt = res.tile([P, NHC, E], bf16, name="wpt")   # W_proj rows
            ot = res.tile([P, NHC, T], f32r, name="ot")     # O^T [dv, h, t]
            ones_f = res.tile([P, 1], f32, name="ones_f")
            nc.vector.memset(ones_f, 1.0)
            ones_bf = res.tile([P, 1], bf16, name="ones_bf")
            nc.vector.tensor_copy(ones_bf, ones_f)
            nc.sync.dma_start(
                wpt, wp.rearrange("(ho p) o -> p ho o", p=P))

            def qkv_rounds(t0, pa_x, pa_w, pa_ps, do_q):
                """K, V (and optionally Q) projection rounds for one 1024-col
                t-block.  Returns the xe tiles (kept alive by pa_x pool)."""
                xe = [None] * NE
                srcs = [(wk, kt), (wv, None)] + ([(wq, qt)] if do_q else [])
                for wsrc, dst in srcs:
                    if dst is not None:
                        # K/Q round: psum[c*2+half] = [c128, t512]
                        pss = [
                            pa_ps.tile([P, 512], f32, name="psqk",
                                       tag="pa_ps", bufs=8)
                            for _ in range(8)
                        ]
                        for e in range(NE):
                            if xe[e] is None:
                                xe[e] = pa_x.tile([P, 1024], f32r,
                                                  name=f"xe{e}",
                                                  tag=f"xe{e}", bufs=1)
                                nc.sync.dma_start(
                                    xe[e],
                                    xT[e * P:(e + 1) * P, t0:t0 + 1024])
                            we = pa_w.tile([P, CQ], f32r, name="we",
                                           tag="we", bufs=3)
                            nc.sync.dma_start(we, wsrc[e * P:(e + 1) * P, :])
                            for c in range(4):
                                for half in range(2):
                                    nc.tensor.matmul(
                                        pss[c * 2 + half],
                                        lhsT=we[:, c * P:(c + 1) * P],
                                        rhs=xe[e][:, half * 512:(half + 1) * 512],
                                        start=(e == 0), stop=(e == NE - 1),
                                    )
                        for c in range(4):
                            for half in range(2):
                                nc.vector.tensor_copy(
                                    dst[:, c, t0 + half * 512:t0 + (half + 1) * 512],
                                    pss[c * 2 + half])
                    else:
                        # V round: psum[tj] = [t128, dv512]
                        psv = [
                            pa_ps.tile([P, 512], f32, name="psv",
                                       tag="pa_ps", bufs=8)
                            for _ in range(8)
                        ]
                        for e in range(NE):
                            we = pa_w.tile([P, CQ], f32r, name="we",
                                           tag="we", bufs=3)
                            nc.sync.dma_start(we, wv[e * P:(e + 1) * P, :])
                            for tj in range(8):
                                nc.tensor.matmul(
                                    psv[tj],
                                    lhsT=xe[e][:, tj * P:(tj + 1) * P],
                                    rhs=we,
                                    start=(e == 0), stop=(e == NE - 1),
                                )
                        for tj in range(8):
                            nc.vector.tensor_copy(
                                vsb[:, (t0 // P) + tj, :], psv[tj])
                return xe

            # ---------- Phase A, t-block 0: K, V, Q ----------
            with (
                tc.tile_pool(name="pa_x0", bufs=1) as pa_x0,
                tc.tile_pool(name="pa_w0", bufs=1) as pa_w0,
                tc.tile_pool(name="pa_ps0", bufs=1, space="PSUM") as pa_ps0,
            ):
                qkv_rounds(0, pa_x0, pa_w0, pa_ps0, do_q=True)

            # ---------- Phase A, t-block 1: K, V (Q deferred) ----------
            with tc.tile_pool(name="pa_x1", bufs=1) as pa_x1:
                with (
                    tc.tile_pool(name="pa_w1", bufs=1) as pa_w1,
                    tc.tile_pool(name="pa_ps1", bufs=1, space="PSUM") as pa_ps1,
                ):
                    xe1 = qkv_rounds(1024, pa_x1, pa_w1, pa_ps1, do_q=False)

                # ---------- Phase B/D: attention + deferred Q1 + out proj ---
                with (
                    tc.tile_pool(name="pb_e", bufs=1) as pb_e,
                    tc.tile_pool(name="pb_r", bufs=1) as pb_r,
                    tc.tile_pool(name="pb_m", bufs=1) as pb_m,
                    tc.tile_pool(name="pd_o", bufs=1) as pd_o,
                    tc.tile_pool(name="ps_a", bufs=1, space="PSUM") as ps_a,
                    tc.tile_pool(name="ps_o", bufs=1, space="PSUM") as ps_o,
                    tc.tile_pool(name="ps_zd", bufs=1, space="PSUM") as ps_zd,
                ):
                    def attn_unit(h, tq):
                        tr = tq * 512
                        pso = ps_o.tile([P, 512], f32, name="pso",
                                        tag="pso", bufs=2)
                        rs = pb_r.tile([P, 512], bf16, name="rs",
                                       tag="rs", bufs=2)
                        for s in range(NS):
                            psa = ps_a.tile([P, 512], f32, name="psa",
                                            tag="psa", bufs=2)
                            nc.tensor.matmul(
                                psa,
                                lhsT=kt[:, h, s * P:(s + 1) * P],
                                rhs=qt[:, h, tr:tr + 512],
                                start=True, stop=True,
                            )
                            et = pb_e.tile([P, 512], bf16, name="et",
                                           tag="et", bufs=4)
                            nc.scalar.activation(et, psa, EXP)
                            nc.tensor.matmul(
                                pso,
                                lhsT=vsb[:, s, h * P:(h + 1) * P],
                                rhs=et,
                                start=(s == 0), stop=(s == NS - 1),
                            )
                            if s == 0:
                                nc.vector.tensor_copy(rs, et)
                            else:
                                nc.vector.tensor_add(rs, rs, et)
                        psz = ps_zd.tile([1, 512], f32, name="psz",
                                         tag="ps_zd", bufs=2)
                        nc.tensor.matmul(psz, lhsT=ones_bf[:, 0:1], rhs=rs,
                                         start=True, stop=True)
                        zr = pb_m.tile([1, 512], f32, name="zr",
                                       tag="zr", bufs=2)
                        nc.vector.reciprocal(zr, psz)
                        rb = pb_m.tile([P, 512], f32, name="rb",
                                       tag="rb", bufs=2)
                        nc.gpsimd.partition_broadcast(rb, zr)
                        nc.vector.tensor_mul(ot[:, h, tr:tr + 512], pso, rb)

                    def d_group(tj, eo):
                        psd = ps_zd.tile([P, 512], f32, name="psd",
                                         tag="ps_zd", bufs=2)
                        for h in range(NHC):
                            nc.tensor.matmul(
                                psd,
                                lhsT=ot[:, h, tj * P:(tj + 1) * P],
                                rhs=wpt[:, h, eo * 512:(eo + 1) * 512],
                                start=(h == 0), stop=(h == NHC - 1),
                            )
                        osb = pd_o.tile([P, 512], f32, name="osb",
                                        tag="osb", bufs=3)
                        nc.vector.tensor_copy(osb, psd)
                        nc.sync.dma_start(
                            out[tj * P:(tj + 1) * P,
                                eo * 512:(eo + 1) * 512],
                            osb)

                    # B t-block 0 (tq 0,1) with Q1 sub-rounds interleaved
                    with (
                        tc.tile_pool(name="pq1w", bufs=1) as pq1w,
                        tc.tile_pool(name="ps_q1", bufs=1,
                                     space="PSUM") as ps_q1,
                    ):
                        for h in range(NHC):
                            attn_unit(h, 0)
                            attn_unit(h, 1)
                            # Q1 sub-round for head column h: 2 psum banks
                            psq = [
                                ps_q1.tile([P, 512], f32, name="psq",
                                           tag="ps_q1", bufs=2)
                                for _ in range(2)
                            ]
                            for e in range(NE):
                                weq = pq1w.tile([P, P], f32r, name="weq",
                                                tag="weq", bufs=3)
                                nc.sync.dma_start(
                                    weq,
                                    wq[e * P:(e + 1) * P, h * P:(h + 1) * P])
                                for half in range(2):
                                    nc.tensor.matmul(
                                        psq[half],
                                        lhsT=weq,
                                        rhs=xe1[e][:, half * 512:(half + 1) * 512],
                                        start=(e == 0), stop=(e == NE - 1),
                                    )
                            for half in range(2):
                                nc.vector.tensor_copy(
                                    qt[:, h, 1024 + half * 512:
                                       1024 + (half + 1) * 512],
                                    psq[half])

                    # B t-block 1 (tq 2,3) with D t-block-0 interleaved
                    for h in range(NHC):
                        attn_unit(h, 2)
                        attn_unit(h, 3)
                        for tj in range(2 * h, 2 * h + 2):
                            for eo in range(4):
                                d_group(tj, eo)

                    # D t-block 1 tail
                    for tj in range(8, 16):
                        for eo in range(4):
                            d_group(tj, eo)

    nc.compile()
    return nc


def _get_nc():
    global _NC_CACHE
    if _NC_CACHE is None:
        _NC_CACHE = _build_nc()
    return _NC_CACHE


def _shard_inputs(x, W_attn, W_proj, lambda_q1, lambda_k1,
                  lambda_q2, lambda_k2):
    import ml_dtypes
    x = np.asarray(x, np.float32)
    W_attn = np.asarray(W_attn, np.float32)
    W_proj = np.asarray(W_proj, np.float32)
    lam = float(np.exp(np.dot(np.asarray(lambda_q1, np.float32),
                              np.asarray(lambda_k1, np.float32)))
                - np.exp(np.dot(np.asarray(lambda_q2, np.float32),
                                np.asarray(lambda_k2, np.float32)))
                + LAMBDA_INIT)
    Cb = E // 2  # 1024: q1/k1/q2/k2 block width in W_attn
    in_maps = []
    for c in range(N_CORES):
        b, hg = divmod(c, 4)
        heads = [4 * hg + j for j in range(NHC)]
        wq_c = np.empty((E, CQ), np.float32)
        wk_c = np.empty((E, CQ), np.float32)
        wv_c = np.empty((E, CQ), np.float32)
        wp_c = np.empty((CQ, E), np.float32)
        for j, h in enumerate(heads):
            wq_c[:, j * P:j * P + HD] = W_attn[:, h * HD:(h + 1) * HD] * SCALE
            wq_c[:, j * P + HD:(j + 1) * P] = (
                W_attn[:, 2 * Cb + h * HD:2 * Cb + (h + 1) * HD]
                * (-lam * SCALE))
            wk_c[:, j * P:j * P + HD] = W_attn[:, Cb + h * HD:Cb + (h + 1) * HD]
            wk_c[:, j * P + HD:(j + 1) * P] = (
                W_attn[:, 3 * Cb + h * HD:3 * Cb + (h + 1) * HD])
            wv_c[:, j * P:(j + 1) * P] = (
                W_attn[:, 4 * Cb + h * DV:4 * Cb + (h + 1) * DV])
            wp_c[j * P:(j + 1) * P, :] = (
                W_proj[h * DV:(h + 1) * DV, :] * (1.0 - LAMBDA_INIT))
        in_maps.append({
            "xT": np.ascontiguousarray(x[b].T),
            "wq": wq_c, "wk": wk_c, "wv": wv_c,
            "wp": wp_c.astype(ml_dtypes.bfloat16),
        })
    return in_maps


def _run(inputs, trace=False):
    from concourse.bass_utils import run_bass_kernel_spmd
    nc = _get_nc()
    in_maps = _shard_inputs(**inputs)
    res = run_bass_kernel_spmd(nc, in_maps, list(range(N_CORES)),
                               trace=trace)
    out = np.zeros((B, T, E), np.float32)
    for c in range(N_CORES):
        out[c // 4] += res.results[c]["out"]
    return out, res


def kernel(x, W_attn, W_proj, lambda_q1, lambda_k1, lambda_q2, lambda_k2):
    out, _ = _run(dict(x=x, W_attn=W_attn, W_proj=W_proj,
                       lambda_q1=lambda_q1, lambda_k1=lambda_k1,
                       lambda_q2=lambda_q2, lambda_k2=lambda_k2))
    return out


# revision 22
# speedup vs baseline: 1.2595x; 1.2093x over previous
"""MultiHeadDiffAttention Trainium2 kernel.

Strategy (8 NeuronCores, SPMD):
  - Shard: batch (B=2) x head-groups (16 heads -> 4 groups of 4).
    Core c handles b = c//4, heads 4*(c%4) .. 4*(c%4)+3.
  - Differential attention is folded into a single 128-dim attention per head:
      q' = [q1 * scale | q2 * (-lam*scale)],  k' = [k1 | k2]
    so logits = scale*(q1k1 - lam*q2k2) come from ONE 128-contraction matmul.
  - Logits are computed transposed (A^T[s,t]) so that exp(A^T) tiles feed the
    O^T = V^T P^T matmul directly (contraction over s on partitions).
  - Softmax denominator Z[t]: instead of a ones-column matmul per exp tile
    (which costs the PE a third 512-cycle pass per score tile, ~55us/core),
    the exp tiles (bf16) are pre-reduced over s on the otherwise-idle DVE
    (running sum), and ONE tiny ones-matmul per 512-col unit finishes the
    partition reduction.  PE attention work drops from 3 to 2 matmuls/tile.
  - Since the attention stretch is then ACT(exp)-bound, PE gaps are filled by
    interleaving independent matmul work in program order: the tb1 Q
    projection runs inside attention-tb0, and the tb0 output projection runs
    inside attention-tb1 (the Tile list scheduler slots them into exp waits).
  - x, W and the q/k/v/exp tensors are bf16 (PE rate is identical, input DMA
    halves, DVE gets its 2x mode; the 2e-2 rel-err budget absorbs it).
  - Per-core output is the head-group's slice of out @ W_proj (row-parallel);
    the host sums the 4 partials per batch element.
"""

import math

import numpy as np

_PSA_BUFS = 3                 # psa PSUM slots (1 bank each)
_QD_BUFS = 2                  # shared Q1/out-proj PSUM slots
_PSO_BUFS = 2                 # pso PSUM slots
_WE_BUFS = 4                  # weight-stream SBUF tiles per round

B, T, E = 2, 2048, 2048
N_HEAD = 16
HD = 64                       # per-component head dim (q1/k1/q2/k2)
DV = 128                      # v head dim
SCALE = HD ** -0.5
LAMBDA_INIT = 0.8 - 0.6 * math.exp(-0.3 * (1 - 1))
P = 128
NHC = 4                       # heads per core
CQ = NHC * DV                 # 512: per-core q'/k'/v width
N_CORES = 8
NE = E // P                   # 16 contraction chunks
NS = T // P                   # 16 s chunks
NTQ = T // 512                # 4 t units of 512 for attention

_NC_CACHE = None


def _build_nc():
    import concourse.mybir as mybir
    import concourse.tile as tile
    from concourse import bacc

    f32 = mybir.dt.float32
    f32r = mybir.dt.float32r
    bf16 = mybir.dt.bfloat16
    EXP = mybir.ActivationFunctionType.Exp

    nc = bacc.Bacc("TRN2", target_bir_lowering=False, debug=False,
                   num_devices=N_CORES)
    xT = nc.dram_tensor("xT", [E, T], bf16, kind="ExternalInput").ap()
    wq = nc.dram_tensor("wq", [E, CQ], bf16, kind="ExternalInput").ap()
    wk = nc.dram_tensor("wk", [E, CQ], bf16, kind="ExternalInput").ap()
    wv = nc.dram_tensor("wv", [E, CQ], bf16, kind="ExternalInput").ap()
    wp = nc.dram_tensor("wp", [CQ, E], bf16, kind="ExternalInput").ap()
    out = nc.dram_tensor("out", [T, E], f32, kind="ExternalOutput").ap()

    with tile.TileContext(nc) as tc:
        with tc.tile_pool(name="res", bufs=1) as res:
            qt = res.tile([P, NHC, T], bf16, name="qt")     # Q'^T [d, h, t]
            kt = res.tile([P, NHC, T], bf16, name="kt")     # K'^T [d, h, s]
            vsb = res.tile([P, NS, CQ], bf16, name="vsb")   # V [t%128, tc, dv]
            wpt = res.tile([P, NHC, E], bf16, name="wpt")   # W_proj rows
            ot = res.tile([P, NHC, T], bf16, name="ot")     # O^T [dv, h, t]
            ones_f = res.tile([P, 1], f32, name="ones_f")
            nc.vector.memset(ones_f, 1.0)
            ones_bf = res.tile([P, 1], bf16, name="ones_bf")
            nc.vector.tensor_copy(ones_bf, ones_f)

            def qkv_rounds(t0, pa_x, pa_w, pa_ps, do_q,
                           xe_pre=None, after_k=None):
                """K, V (and optionally Q) projection rounds for one 1024-col
                t-block.  Returns the xe tiles (kept alive by pa_x pool)."""
                xe = xe_pre if xe_pre is not None else [None] * NE
                srcs = [(wk, kt, "wek"), (wv, None, "wev")] + (
                    [(wq, qt, "weq0")] if do_q else [])
                for wsrc, dst, wtag in srcs:
                    if dst is not None:
                        # K/Q round: psum[c*2+half] = [c128, t512]
                        pss = [
                            pa_ps.tile([P, 512], f32, name="psqk",
                                       tag="pa_ps", bufs=8)
                            for _ in range(8)
                        ]
                        for e in range(NE):
                            if xe[e] is None:
                                xe[e] = pa_x.tile([P, 1024], bf16,
                                                  name=f"xe{e}",
                                                  tag=f"xe{e}", bufs=1)
                                nc.sync.dma_start(
                                    xe[e],
                                    xT[e * P:(e + 1) * P, t0:t0 + 1024])
                            we = pa_w.tile([P, CQ], bf16, name="we",
                                           tag=wtag, bufs=_WE_BUFS)
                            nc.sync.dma_start(we, wsrc[e * P:(e + 1) * P, :])
                            for c in range(4):
                                for half in range(2):
                                    nc.tensor.matmul(
                                        pss[c * 2 + half],
                                        lhsT=we[:, c * P:(c + 1) * P],
                                        rhs=xe[e][:, half * 512:(half + 1) * 512],
                                        start=(e == 0), stop=(e == NE - 1),
                                    )
                        for c in range(4):
                            for half in range(2):
                                eng = nc.vector.tensor_copy if half else \
                                    nc.scalar.copy
                                eng(
                                    dst[:, c, t0 + half * 512:t0 + (half + 1) * 512],
                                    pss[c * 2 + half])
                        if after_k is not None:
                            after_k()
                            after_k = None
                    else:
                        # V round: psum[tj] = [t128, dv512]
                        psv = [
                            pa_ps.tile([P, 512], f32, name="psv",
                                       tag="pa_ps", bufs=8)
                            for _ in range(8)
                        ]
                        for e in range(NE):
                            we = pa_w.tile([P, CQ], bf16, name="we",
                                           tag=wtag, bufs=_WE_BUFS)
                            nc.sync.dma_start(we, wv[e * P:(e + 1) * P, :])
                            for tj in range(8):
                                nc.tensor.matmul(
                                    psv[tj],
                                    lhsT=xe[e][:, tj * P:(tj + 1) * P],
                                    rhs=we,
                                    start=(e == 0), stop=(e == NE - 1),
                                )
                        for tj in range(8):
                            eng = nc.vector.tensor_copy if tj % 2 else \
                                nc.scalar.copy
                            eng(vsb[:, (t0 // P) + tj, :], psv[tj])
                return xe

            # ---------- Phase A, t-block 0: K, V, Q ----------
            with (
                tc.tile_pool(name="pa_x0", bufs=1) as pa_x0,
                tc.tile_pool(name="pa_w0", bufs=1) as pa_w0,
                tc.tile_pool(name="pa_ps0", bufs=1, space="PSUM") as pa_ps0,
            ):
                qkv_rounds(0, pa_x0, pa_w0, pa_ps0, do_q=True)

            # ---------- Phase A, t-block 1: K, V (Q deferred) ----------
            with tc.tile_pool(name="pa_x1", bufs=1) as pa_x1:
                with (
                    tc.tile_pool(name="pa_w1", bufs=1) as pa_w1,
                    tc.tile_pool(name="pa_ps1", bufs=1, space="PSUM") as pa_ps1,
                ):
                    xe1 = qkv_rounds(1024, pa_x1, pa_w1, pa_ps1, do_q=False)
                    # prefetch the projection weights during attention tb0
                    nc.sync.dma_start(
                        wpt, wp.rearrange("(ho p) o -> p ho o", p=P))

                # ---------- Phase B/D: attention + deferred Q1 + out proj ---
                # PE gaps inside the ACT(exp)-bound attention iterations are
                # filled explicitly: one matmul from a "filler" generator
                # (deferred Q1 projection during t-block 0, tb0 output
                # projection during t-block 1) is emitted after each
                # attention iteration, so PE per-iter work (psa+pso+fill)
                # matches the exp cadence instead of idling.
                with (
                    tc.tile_pool(name="pb_e", bufs=1) as pb_e,
                    tc.tile_pool(name="pb_r", bufs=1) as pb_r,
                    tc.tile_pool(name="pb_m", bufs=1) as pb_m,
                    tc.tile_pool(name="pd_o", bufs=1) as pd_o,
                    tc.tile_pool(name="pq1w", bufs=1) as pq1w,
                    tc.tile_pool(name="ps_a", bufs=1, space="PSUM") as ps_a,
                    tc.tile_pool(name="ps_o", bufs=1, space="PSUM") as ps_o,
                    tc.tile_pool(name="ps_z", bufs=1, space="PSUM") as ps_z,
                    tc.tile_pool(name="ps_qd", bufs=1, space="PSUM") as ps_qd,
                ):
                    def attn_unit(h, tq, filler):
                        tr = tq * 512
                        pso = ps_o.tile([P, 512], f32, name="pso",
                                        tag="pso", bufs=_PSO_BUFS)
                        rs = pb_r.tile([P, 512], bf16, name="rs",
                                       tag="rs", bufs=3)
                        for s in range(NS):
                            psa = ps_a.tile([P, 512], f32, name="psa",
                                            tag="psa", bufs=_PSA_BUFS)
                            nc.tensor.matmul(
                                psa,
                                lhsT=kt[:, h, s * P:(s + 1) * P],
                                rhs=qt[:, h, tr:tr + 512],
                                start=True, stop=True,
                            )
                            et = pb_e.tile([P, 512], bf16, name="et",
                                           tag="et", bufs=6)
                            nc.scalar.activation(et, psa, EXP)
                            next(filler, None)
                            nc.tensor.matmul(
                                pso,
                                lhsT=vsb[:, s, h * P:(h + 1) * P],
                                rhs=et,
                                start=(s == 0), stop=(s == NS - 1),
                            )
                            if s == 0:
                                et_prev = et
                            elif s == 1:
                                nc.vector.tensor_add(rs, et_prev, et)
                            else:
                                nc.vector.tensor_add(rs, rs, et)
                        psz = ps_z.tile([1, 512], f32, name="psz",
                                        tag="ps_z", bufs=1)
                        nc.tensor.matmul(psz, lhsT=ones_bf[:, 0:1], rhs=rs,
                                         start=True, stop=True)
                        zr = pb_m.tile([1, 512], f32, name="zr",
                                       tag="zr", bufs=2)
                        nc.vector.reciprocal(zr, psz)
                        rb = pb_m.tile([P, 512], f32, name="rb",
                                       tag="rb", bufs=2)
                        nc.gpsimd.partition_broadcast(rb, zr)
                        nc.vector.tensor_mul(ot[:, h, tr:tr + 512], pso, rb)

                    def gen_q1():
                        # Deferred tb1 Q projection, one matmul per yield.
                        for c in range(NHC):
                            psq = [
                                ps_qd.tile([P, 512], f32, name="psq",
                                           tag="ps_qd", bufs=_QD_BUFS)
                                for _ in range(2)
                            ]
                            for e in range(NE):
                                weq = pq1w.tile([P, P], bf16, name="weq",
                                                tag="weq", bufs=3)
                                nc.sync.dma_start(
                                    weq,
                                    wq[e * P:(e + 1) * P, c * P:(c + 1) * P])
                                for half in range(2):
                                    nc.tensor.matmul(
                                        psq[half],
                                        lhsT=weq,
                                        rhs=xe1[e][:, half * 512:(half + 1) * 512],
                                        start=(e == 0), stop=(e == NE - 1),
                                    )
                                    yield
                            for half in range(2):
                                nc.vector.tensor_copy(
                                    qt[:, c, 1024 + half * 512:
                                       1024 + (half + 1) * 512],
                                    psq[half])

                    def gen_d(tj0, tj1, tail=False):
                        # Output projection for t rows tj0*128..tj1*128,
                        # one matmul per yield.
                        for tj in range(tj0, tj1):
                            for eo in range(4):
                                if tail and eo % 2:
                                    psd = ps_a.tile([P, 512], f32,
                                                    name="psd_t", tag="psa",
                                                    bufs=_PSA_BUFS)
                                else:
                                    psd = ps_qd.tile([P, 512], f32,
                                                     name="psd",
                                                     tag="ps_qd",
                                                     bufs=_QD_BUFS)
                                for h in range(NHC):
                                    nc.tensor.matmul(
                                        psd,
                                        lhsT=ot[:, h, tj * P:(tj + 1) * P],
                                        rhs=wpt[:, h,
                                                eo * 512:(eo + 1) * 512],
                                        start=(h == 0), stop=(h == NHC - 1),
                                    )
                                    yield
                                osb = pd_o.tile([P, 512], f32, name="osb",
                                                tag="osb", bufs=4)
                                if tail:
                                    nc.scalar.copy(osb, psd)
                                else:
                                    nc.vector.tensor_copy(osb, psd)
                                nc.sync.dma_start(
                                    out[tj * P:(tj + 1) * P,
                                        eo * 512:(eo + 1) * 512],
                                    osb)

                    def drain(gen):
                        for _ in gen:
                            pass

                    # B t-block 0 (tq 0,1): interleave the deferred Q1
                    # projection (128 matmuls over 128 attention iters).
                    q1 = gen_q1()
                    for h in range(NHC):
                        attn_unit(h, 0, q1)
                        attn_unit(h, 1, q1)
                    drain(q1)

                    # B t-block 1: interleave the tb0 output projection,
                    # then (once the tq2 units complete) the first half of
                    # the tb1 projection, at a uniform 1.5 fills per
                    # attention iter -- this balances PE per-iter work
                    # against the exp cadence across the whole block.
                    def chain2(*gens):
                        for g in gens:
                            yield from g

                    def pace(gen, per2):
                        # yields once per attention iter, pulling per2
                        # fills every 2 iters from gen
                        flip = False
                        while True:
                            for _ in range(per2 // 2 + (1 if flip and
                                           per2 % 2 else 0)):
                                next(gen, None)
                            flip = not flip
                            yield

                    d1a = chain2(gen_d(0, 8), gen_d(8, 12))
                    d_paced = pace(d1a, 3)
                    for h in range(NHC):
                        attn_unit(h, 2, d_paced)
                    for h in range(NHC):
                        attn_unit(h, 3, d_paced)
                    drain(d1a)

                    # D tail: remaining quarter of the tb1 projection
                    drain(gen_d(12, 16, tail=True))

    nc.compile()
    return nc


def _get_nc():
    global _NC_CACHE
    if _NC_CACHE is None:
        _NC_CACHE = _build_nc()
    return _NC_CACHE


def _shard_inputs(x, W_attn, W_proj, lambda_q1, lambda_k1,
                  lambda_q2, lambda_k2):
    import ml_dtypes
    x = np.asarray(x, np.float32)
    W_attn = np.asarray(W_attn, np.float32)
    W_proj = np.asarray(W_proj, np.float32)
    lam = float(np.exp(np.dot(np.asarray(lambda_q1, np.float32),
                              np.asarray(lambda_k1, np.float32)))
                - np.exp(np.dot(np.asarray(lambda_q2, np.float32),
                                np.asarray(lambda_k2, np.float32)))
                + LAMBDA_INIT)
    Cb = E // 2  # 1024: q1/k1/q2/k2 block width in W_attn
    in_maps = []
    for c in range(N_CORES):
        b, hg = divmod(c, 4)
        heads = [4 * hg + j for j in range(NHC)]
        wq_c = np.empty((E, CQ), np.float32)
        wk_c = np.empty((E, CQ), np.float32)
        wv_c = np.empty((E, CQ), np.float32)
        wp_c = np.empty((CQ, E), np.float32)
        for j, h in enumerate(heads):
            wq_c[:, j * P:j * P + HD] = W_attn[:, h * HD:(h + 1) * HD] * SCALE
            wq_c[:, j * P + HD:(j + 1) * P] = (
                W_attn[:, 2 * Cb + h * HD:2 * Cb + (h + 1) * HD]
                * (-lam * SCALE))
            wk_c[:, j * P:j * P + HD] = W_attn[:, Cb + h * HD:Cb + (h + 1) * HD]
            wk_c[:, j * P + HD:(j + 1) * P] = (
                W_attn[:, 3 * Cb + h * HD:3 * Cb + (h + 1) * HD])
            wv_c[:, j * P:(j + 1) * P] = (
                W_attn[:, 4 * Cb + h * DV:4 * Cb + (h + 1) * DV])
            wp_c[j * P:(j + 1) * P, :] = (
                W_proj[h * DV:(h + 1) * DV, :] * (1.0 - LAMBDA_INIT))
        bf = ml_dtypes.bfloat16
        in_maps.append({
            "xT": np.ascontiguousarray(x[b].T).astype(bf),
            "wq": wq_c.astype(bf), "wk": wk_c.astype(bf),
            "wv": wv_c.astype(bf),
            "wp": wp_c.astype(bf),
        })
    return in_maps


def _run(inputs, trace=False):
    from concourse.bass_utils import run_bass_kernel_spmd
    nc = _get_nc()
    in_maps = _shard_inputs(**inputs)
    res = run_bass_kernel_spmd(nc, in_maps, list(range(N_CORES)),
                               trace=trace)
    out = np.zeros((B, T, E), np.float32)
    for c in range(N_CORES):
        out[c // 4] += res.results[c]["out"]
    return out, res


def kernel(x, W_attn, W_proj, lambda_q1, lambda_k1, lambda_q2, lambda_k2):
    out, _ = _run(dict(x=x, W_attn=W_attn, W_proj=W_proj,
                       lambda_q1=lambda_q1, lambda_k1=lambda_k1,
                       lambda_q2=lambda_q2, lambda_k2=lambda_k2))
    return out



# revision 30
# speedup vs baseline: 1.2882x; 1.0228x over previous
"""MultiHeadDiffAttention Trainium2 kernel.

Strategy (8 NeuronCores, SPMD):
  - Shard: batch (B=2) x head-groups (16 heads -> 4 groups of 4).
    Core c handles b = c//4, heads 4*(c%4) .. 4*(c%4)+3.
  - Differential attention is folded into a single 128-dim attention per head:
      q' = [q1 * scale | q2 * (-lam*scale)],  k' = [k1 | k2]
    so logits = scale*(q1k1 - lam*q2k2) come from ONE 128-contraction matmul.
  - Logits are computed transposed (A^T[s,t]) so that exp(A^T) tiles feed the
    O^T = V^T P^T matmul directly (contraction over s on partitions).
  - Softmax denominator Z[t]: instead of a ones-column matmul per exp tile
    (which costs the PE a third 512-cycle pass per score tile, ~55us/core),
    the exp tiles (bf16) are pre-reduced over s on the otherwise-idle DVE
    (running sum), and ONE tiny ones-matmul per 512-col unit finishes the
    partition reduction.  PE attention work drops from 3 to 2 matmuls/tile.
  - Since the attention stretch is then ACT(exp)-bound, PE gaps are filled by
    interleaving independent matmul work in program order: the tb1 Q
    projection runs inside attention-tb0, and the tb0 output projection runs
    inside attention-tb1 (the Tile list scheduler slots them into exp waits).
  - x, W and the q/k/v/exp tensors are bf16 (PE rate is identical, input DMA
    halves, DVE gets its 2x mode; the 2e-2 rel-err budget absorbs it).
  - Per-core output is the head-group's slice of out @ W_proj (row-parallel);
    the host sums the 4 partials per batch element.
"""

import math

import numpy as np

_PSA_BUFS = 3                 # psa PSUM slots (1 bank each)
_QD_BUFS = 2                  # shared Q1/out-proj PSUM slots
_PSO_BUFS = 2                 # pso PSUM slots
_WE_BUFS = 4                  # weight-stream SBUF tiles per round

B, T, E = 2, 2048, 2048
N_HEAD = 16
HD = 64                       # per-component head dim (q1/k1/q2/k2)
DV = 128                      # v head dim
SCALE = HD ** -0.5
LAMBDA_INIT = 0.8 - 0.6 * math.exp(-0.3 * (1 - 1))
P = 128
NHC = 4                       # heads per core
CQ = NHC * DV                 # 512: per-core q'/k'/v width
N_CORES = 8
NE = E // P                   # 16 contraction chunks
NS = T // P                   # 16 s chunks
NTQ = T // 512                # 4 t units of 512 for attention

_NC_CACHE = None


def _build_nc():
    import concourse.mybir as mybir
    import concourse.tile as tile
    from concourse import bacc

    f32 = mybir.dt.float32
    f32r = mybir.dt.float32r
    bf16 = mybir.dt.bfloat16
    EXP = mybir.ActivationFunctionType.Exp

    nc = bacc.Bacc("TRN2", target_bir_lowering=False, debug=False,
                   num_devices=N_CORES)
    xT = nc.dram_tensor("xT", [E, T], bf16, kind="ExternalInput").ap()
    wq = nc.dram_tensor("wq", [E, CQ], bf16, kind="ExternalInput").ap()
    wk = nc.dram_tensor("wk", [E, CQ], bf16, kind="ExternalInput").ap()
    wv = nc.dram_tensor("wv", [E, CQ], bf16, kind="ExternalInput").ap()
    wp = nc.dram_tensor("wp", [CQ, E], bf16, kind="ExternalInput").ap()
    out = nc.dram_tensor("out", [T, E], bf16, kind="ExternalOutput").ap()

    with tile.TileContext(nc) as tc:
        with tc.tile_pool(name="res", bufs=1) as res:
            qt = res.tile([P, NHC, T], bf16, name="qt")     # Q'^T [d, h, t]
            kt = res.tile([P, NHC, T], bf16, name="kt")     # K'^T [d, h, s]
            vsb = res.tile([P, NS, CQ], bf16, name="vsb")   # V [t%128, tc, dv]
            wpt = res.tile([P, NHC, E], bf16, name="wpt")   # W_proj rows
            ot = res.tile([P, NHC, T], bf16, name="ot")     # O^T [dv, h, t]
            wtile = res.tile([P, 512], bf16, name="wtile")
            nc.vector.memset(wtile, 0.0)
            ones_f = res.tile([P, 1], f32, name="ones_f")
            nc.vector.memset(ones_f, 1.0)
            ones_bf = res.tile([P, 1], bf16, name="ones_bf")
            nc.vector.tensor_copy(ones_bf, ones_f)

            def qkv_rounds(t0, pa_x, pa_w, pa_ps, do_q,
                           xe_pre=None, after_k=None):
                """K, V (and optionally Q) projection rounds for one 1024-col
                t-block.  Returns the xe tiles (kept alive by pa_x pool)."""
                xe = xe_pre if xe_pre is not None else [None] * NE
                srcs = [(wk, kt, "wek"), (wv, None, "wev")] + (
                    [(wq, qt, "weq0")] if do_q else [])
                for wsrc, dst, wtag in srcs:
                    if dst is not None:
                        # K/Q round: psum[c*2+half] = [c128, t512]
                        pss = [
                            pa_ps.tile([P, 512], f32, name="psqk",
                                       tag="pa_ps", bufs=8)
                            for _ in range(8)
                        ]
                        for e in range(NE):
                            if xe[e] is None:
                                xe[e] = pa_x.tile([P, 1024], bf16,
                                                  name=f"xe{e}",
                                                  tag=f"xe{e}", bufs=1)
                                nc.sync.dma_start(
                                    xe[e],
                                    xT[e * P:(e + 1) * P, t0:t0 + 1024])
                            we = pa_w.tile([P, CQ], bf16, name="we",
                                           tag=wtag, bufs=_WE_BUFS)
                            nc.sync.dma_start(we, wsrc[e * P:(e + 1) * P, :])
                            for c in range(4):
                                for half in range(2):
                                    nc.tensor.matmul(
                                        pss[c * 2 + half],
                                        lhsT=we[:, c * P:(c + 1) * P],
                                        rhs=xe[e][:, half * 512:(half + 1) * 512],
                                        start=(e == 0), stop=(e == NE - 1),
                                    )
                        for c in range(4):
                            for half in range(2):
                                eng = nc.vector.tensor_copy if half else \
                                    nc.scalar.copy
                                eng(
                                    dst[:, c, t0 + half * 512:t0 + (half + 1) * 512],
                                    pss[c * 2 + half])
                        if after_k is not None:
                            after_k()
                            after_k = None
                    else:
                        # V round: psum[tj] = [t128, dv512]
                        psv = [
                            pa_ps.tile([P, 512], f32, name="psv",
                                       tag="pa_ps", bufs=8)
                            for _ in range(8)
                        ]
                        for e in range(NE):
                            we = pa_w.tile([P, CQ], bf16, name="we",
                                           tag=wtag, bufs=_WE_BUFS)
                            nc.sync.dma_start(we, wv[e * P:(e + 1) * P, :])
                            for tj in range(8):
                                nc.tensor.matmul(
                                    psv[tj],
                                    lhsT=xe[e][:, tj * P:(tj + 1) * P],
                                    rhs=we,
                                    start=(e == 0), stop=(e == NE - 1),
                                )
                        for tj in range(8):
                            eng = nc.vector.tensor_copy if tj % 2 else \
                                nc.scalar.copy
                            eng(vsb[:, (t0 // P) + tj, :], psv[tj])
                return xe

            # ---------- Phase A, t-block 0: K, V, Q ----------
            with (
                tc.tile_pool(name="pa_x0", bufs=1) as pa_x0,
                tc.tile_pool(name="pa_w0", bufs=1) as pa_w0,
                tc.tile_pool(name="pa_ps0", bufs=1, space="PSUM") as pa_ps0,
            ):
                # PE warm-up on memset data while the first x/W DMAs land:
                # completes the p-state ramp so real matmuls run full-speed
                wps = pa_ps0.tile([P, 512], f32, name="wups", tag="pa_ps",
                                  bufs=8)
                for _ in range(4):
                    nc.tensor.matmul(wps, lhsT=wtile[:, 0:P], rhs=wtile,
                                     start=True, stop=True)
                qkv_rounds(0, pa_x0, pa_w0, pa_ps0, do_q=True)

            # ---------- Phase A, t-block 1: K, V (Q deferred) ----------
            with tc.tile_pool(name="pa_x1", bufs=1) as pa_x1:
                with (
                    tc.tile_pool(name="pa_w1", bufs=1) as pa_w1,
                    tc.tile_pool(name="pa_ps1", bufs=1, space="PSUM") as pa_ps1,
                ):
                    wps1 = pa_ps1.tile([P, 512], f32, name="wups1",
                                       tag="pa_ps", bufs=8)
                    for _ in range(4):
                        nc.tensor.matmul(wps1, lhsT=wtile[:, 0:P],
                                         rhs=wtile, start=True, stop=True)
                    xe1 = qkv_rounds(1024, pa_x1, pa_w1, pa_ps1, do_q=False)
                    # prefetch the projection weights during attention tb0
                    nc.sync.dma_start(
                        wpt, wp.rearrange("(ho p) o -> p ho o", p=P))

                # ---------- Phase B/D: attention + deferred Q1 + out proj ---
                # PE gaps inside the ACT(exp)-bound attention iterations are
                # filled explicitly: one matmul from a "filler" generator
                # (deferred Q1 projection during t-block 0, tb0 output
                # projection during t-block 1) is emitted after each
                # attention iteration, so PE per-iter work (psa+pso+fill)
                # matches the exp cadence instead of idling.
                with (
                    tc.tile_pool(name="pb_e", bufs=1) as pb_e,
                    tc.tile_pool(name="pb_r", bufs=1) as pb_r,
                    tc.tile_pool(name="pb_m", bufs=1) as pb_m,
                    tc.tile_pool(name="pd_o", bufs=1) as pd_o,
                    tc.tile_pool(name="pq1w", bufs=1) as pq1w,
                    tc.tile_pool(name="ps_a", bufs=1, space="PSUM") as ps_a,
                    tc.tile_pool(name="ps_o", bufs=1, space="PSUM") as ps_o,
                    tc.tile_pool(name="ps_z", bufs=1, space="PSUM") as ps_z,
                    tc.tile_pool(name="ps_qd", bufs=1, space="PSUM") as ps_qd,
                ):
                    def attn_unit(h, tq, filler):
                        tr = tq * 512
                        pso = ps_o.tile([P, 512], f32, name="pso",
                                        tag="pso", bufs=_PSO_BUFS)
                        rs = pb_r.tile([P, 512], bf16, name="rs",
                                       tag="rs", bufs=3)
                        for s in range(NS):
                            psa = ps_a.tile([P, 512], f32, name="psa",
                                            tag="psa", bufs=_PSA_BUFS)
                            nc.tensor.matmul(
                                psa,
                                lhsT=kt[:, h, s * P:(s + 1) * P],
                                rhs=qt[:, h, tr:tr + 512],
                                start=True, stop=True,
                            )
                            et = pb_e.tile([P, 512], bf16, name="et",
                                           tag="et", bufs=6)
                            nc.scalar.activation(et, psa, EXP)
                            next(filler, None)
                            nc.tensor.matmul(
                                pso,
                                lhsT=vsb[:, s, h * P:(h + 1) * P],
                                rhs=et,
                                start=(s == 0), stop=(s == NS - 1),
                            )
                            if s == 0:
                                et_prev = et
                            elif s == 1:
                                nc.vector.tensor_add(rs, et_prev, et)
                            else:
                                nc.vector.tensor_add(rs, rs, et)
                        psz = ps_z.tile([1, 512], f32, name="psz",
                                        tag="ps_z", bufs=1)
                        nc.tensor.matmul(psz, lhsT=ones_bf[:, 0:1], rhs=rs,
                                         start=True, stop=True)
                        zr = pb_m.tile([1, 512], f32, name="zr",
                                       tag="zr", bufs=2)
                        nc.vector.reciprocal(zr, psz)
                        rb = pb_m.tile([P, 512], f32, name="rb",
                                       tag="rb", bufs=2)
                        nc.gpsimd.partition_broadcast(rb, zr)
                        nc.vector.tensor_mul(ot[:, h, tr:tr + 512], pso, rb)

                    def gen_q1():
                        # Deferred tb1 Q projection, one matmul per yield.
                        for c in range(NHC):
                            psq = [
                                ps_qd.tile([P, 512], f32, name="psq",
                                           tag="ps_qd", bufs=_QD_BUFS)
                                for _ in range(2)
                            ]
                            for e in range(NE):
                                weq = pq1w.tile([P, P], bf16, name="weq",
                                                tag="weq", bufs=3)
                                nc.sync.dma_start(
                                    weq,
                                    wq[e * P:(e + 1) * P, c * P:(c + 1) * P])
                                for half in range(2):
                                    nc.tensor.matmul(
                                        psq[half],
                                        lhsT=weq,
                                        rhs=xe1[e][:, half * 512:(half + 1) * 512],
                                        start=(e == 0), stop=(e == NE - 1),
                                    )
                                    yield
                            for half in range(2):
                                nc.vector.tensor_copy(
                                    qt[:, c, 1024 + half * 512:
                                       1024 + (half + 1) * 512],
                                    psq[half])

                    def gen_d(tj0, tj1, tail=False):
                        # Output projection for t rows tj0*128..tj1*128,
                        # one matmul per yield.
                        for tj in range(tj0, tj1):
                            for eo in range(4):
                                if tail and eo % 2:
                                    psd = ps_a.tile([P, 512], f32,
                                                    name="psd_t", tag="psa",
                                                    bufs=_PSA_BUFS)
                                else:
                                    psd = ps_qd.tile([P, 512], f32,
                                                     name="psd",
                                                     tag="ps_qd",
                                                     bufs=_QD_BUFS)
                                for h in range(NHC):
                                    nc.tensor.matmul(
                                        psd,
                                        lhsT=ot[:, h, tj * P:(tj + 1) * P],
                                        rhs=wpt[:, h,
                                                eo * 512:(eo + 1) * 512],
                                        start=(h == 0), stop=(h == NHC - 1),
                                    )
                                    yield
                                osb = pd_o.tile([P, 512], f32, name="osb",
                                                tag="osb", bufs=4)
                                if tail:
                                    nc.scalar.copy(osb, psd)
                                else:
                                    nc.vector.tensor_copy(osb, psd)
                                nc.sync.dma_start(
                                    out[tj * P:(tj + 1) * P,
                                        eo * 512:(eo + 1) * 512],
                                    osb)

                    def drain(gen):
                        for _ in gen:
                            pass

                    # B t-block 0 (tq 0,1): interleave the deferred Q1
                    # projection (128 matmuls over 128 attention iters).
                    q1 = gen_q1()
                    for h in range(NHC):
                        attn_unit(h, 0, q1)
                        attn_unit(h, 1, q1)
                    drain(q1)

                    # B t-block 1: interleave the tb0 output projection,
                    # then (once the tq2 units complete) the first half of
                    # the tb1 projection, at a uniform 1.5 fills per
                    # attention iter -- this balances PE per-iter work
                    # against the exp cadence across the whole block.
                    def chain2(*gens):
                        for g in gens:
                            yield from g

                    def pace(gen, per2):
                        # yields once per attention iter, pulling per2
                        # fills every 2 iters from gen
                        flip = False
                        while True:
                            for _ in range(per2 // 2 + (1 if flip and
                                           per2 % 2 else 0)):
                                next(gen, None)
                            flip = not flip
                            yield

                    d1a = chain2(gen_d(0, 8), gen_d(8, 12))
                    d_paced = pace(d1a, 3)
                    for h in range(NHC):
                        attn_unit(h, 2, d_paced)
                    for h in range(NHC):
                        attn_unit(h, 3, d_paced)
                    drain(d1a)

                    # D tail: remaining quarter of the tb1 projection
                    drain(gen_d(12, 16, tail=True))

    nc.compile()
    return nc


def _get_nc():
    global _NC_CACHE
    if _NC_CACHE is None:
        _NC_CACHE = _build_nc()
    return _NC_CACHE


def _shard_inputs(x, W_attn, W_proj, lambda_q1, lambda_k1,
                  lambda_q2, lambda_k2):
    import ml_dtypes
    x = np.asarray(x, np.float32)
    W_attn = np.asarray(W_attn, np.float32)
    W_proj = np.asarray(W_proj, np.float32)
    lam = float(np.exp(np.dot(np.asarray(lambda_q1, np.float32),
                              np.asarray(lambda_k1, np.float32)))
                - np.exp(np.dot(np.asarray(lambda_q2, np.float32),
                                np.asarray(lambda_k2, np.float32)))
                + LAMBDA_INIT)
    Cb = E // 2  # 1024: q1/k1/q2/k2 block width in W_attn
    in_maps = []
    for c in range(N_CORES):
        b, hg = divmod(c, 4)
        heads = [4 * hg + j for j in range(NHC)]
        wq_c = np.empty((E, CQ), np.float32)
        wk_c = np.empty((E, CQ), np.float32)
        wv_c = np.empty((E, CQ), np.float32)
        wp_c = np.empty((CQ, E), np.float32)
        for j, h in enumerate(heads):
            wq_c[:, j * P:j * P + HD] = W_attn[:, h * HD:(h + 1) * HD] * SCALE
            wq_c[:, j * P + HD:(j + 1) * P] = (
                W_attn[:, 2 * Cb + h * HD:2 * Cb + (h + 1) * HD]
                * (-lam * SCALE))
            wk_c[:, j * P:j * P + HD] = W_attn[:, Cb + h * HD:Cb + (h + 1) * HD]
            wk_c[:, j * P + HD:(j + 1) * P] = (
                W_attn[:, 3 * Cb + h * HD:3 * Cb + (h + 1) * HD])
            wv_c[:, j * P:(j + 1) * P] = (
                W_attn[:, 4 * Cb + h * DV:4 * Cb + (h + 1) * DV])
            wp_c[j * P:(j + 1) * P, :] = (
                W_proj[h * DV:(h + 1) * DV, :] * (1.0 - LAMBDA_INIT))
        bf = ml_dtypes.bfloat16
        in_maps.append({
            "xT": np.ascontiguousarray(x[b].T).astype(bf),
            "wq": wq_c.astype(bf), "wk": wk_c.astype(bf),
            "wv": wv_c.astype(bf),
            "wp": wp_c.astype(bf),
        })
    return in_maps


def _run(inputs, trace=False):
    from concourse.bass_utils import run_bass_kernel_spmd
    nc = _get_nc()
    in_maps = _shard_inputs(**inputs)
    res = run_bass_kernel_spmd(nc, in_maps, list(range(N_CORES)),
                               trace=trace)
    out = np.zeros((B, T, E), np.float32)
    for c in range(N_CORES):
        out[c // 4] += np.asarray(res.results[c]["out"], np.float32)
    return out, res


def kernel(x, W_attn, W_proj, lambda_q1, lambda_k1, lambda_q2, lambda_k2):
    out, _ = _run(dict(x=x, W_attn=W_attn, W_proj=W_proj,
                       lambda_q1=lambda_q1, lambda_k1=lambda_k1,
                       lambda_q2=lambda_q2, lambda_k2=lambda_k2))
    return out



# revision 35
# speedup vs baseline: 1.3330x; 1.0348x over previous
"""MultiHeadDiffAttention Trainium2 kernel.

Strategy (8 NeuronCores, SPMD):
  - Shard: batch (B=2) x head-groups (16 heads -> 4 groups of 4).
    Core c handles b = c//4, heads 4*(c%4) .. 4*(c%4)+3.
  - Differential attention is folded into a single 128-dim attention per head:
      q' = [q1 * scale | q2 * (-lam*scale)],  k' = [k1 | k2]
    so logits = scale*(q1k1 - lam*q2k2) come from ONE 128-contraction matmul.
  - Logits are computed transposed (A^T[s,t]) so that exp(A^T) tiles feed the
    O^T = V^T P^T matmul directly (contraction over s on partitions).
  - Softmax denominator Z[t]: instead of a ones-column matmul per exp tile
    (which costs the PE a third 512-cycle pass per score tile, ~55us/core),
    the exp tiles (bf16) are pre-reduced over s on the otherwise-idle DVE
    (running sum), and ONE tiny ones-matmul per 512-col unit finishes the
    partition reduction.  PE attention work drops from 3 to 2 matmuls/tile.
  - Since the attention stretch is then ACT(exp)-bound, PE gaps are filled by
    interleaving independent matmul work in program order: the tb1 Q
    projection runs inside attention-tb0, and the tb0 output projection runs
    inside attention-tb1 (the Tile list scheduler slots them into exp waits).
  - x, W and the q/k/v/exp tensors are bf16 (PE rate is identical, input DMA
    halves, DVE gets its 2x mode; the 2e-2 rel-err budget absorbs it).
  - Per-core output is the head-group's slice of out @ W_proj (row-parallel);
    the host sums the 4 partials per batch element.
"""

import math

import numpy as np

_PSA_BUFS = 3                 # psa PSUM slots (1 bank each)
_QD_BUFS = 2                  # shared Q1/out-proj PSUM slots
_PSO_BUFS = 2                 # pso PSUM slots
_WE_BUFS = 4                  # weight-stream SBUF tiles per round

B, T, E = 2, 2048, 2048
N_HEAD = 16
HD = 64                       # per-component head dim (q1/k1/q2/k2)
DV = 128                      # v head dim
SCALE = HD ** -0.5
LAMBDA_INIT = 0.8 - 0.6 * math.exp(-0.3 * (1 - 1))
P = 128
NHC = 4                       # heads per core
CQ = NHC * DV                 # 512: per-core q'/k'/v width
N_CORES = 8
NE = E // P                   # 16 contraction chunks
NS = T // P                   # 16 s chunks
NTQ = T // 512                # 4 t units of 512 for attention

_NC_CACHE = None


def _build_nc():
    import concourse.mybir as mybir
    import concourse.tile as tile
    from concourse import bacc

    f32 = mybir.dt.float32
    f32r = mybir.dt.float32r
    bf16 = mybir.dt.bfloat16
    EXP = mybir.ActivationFunctionType.Exp

    nc = bacc.Bacc("TRN2", target_bir_lowering=False, debug=False,
                   num_devices=N_CORES)
    xT = nc.dram_tensor("xT", [E, T], bf16, kind="ExternalInput").ap()
    wq = nc.dram_tensor("wq", [E, CQ], bf16, kind="ExternalInput").ap()
    wk = nc.dram_tensor("wk", [E, CQ], bf16, kind="ExternalInput").ap()
    wv = nc.dram_tensor("wv", [E, CQ], bf16, kind="ExternalInput").ap()
    wp = nc.dram_tensor("wp", [CQ, E], bf16, kind="ExternalInput").ap()
    out = nc.dram_tensor("out", [T, E], bf16, kind="ExternalOutput").ap()

    with tile.TileContext(nc) as tc:
        with tc.tile_pool(name="res", bufs=1) as res:
            qt = res.tile([P, NHC, T], bf16, name="qt")     # Q'^T [d, h, t]
            kt = res.tile([P, NHC, T], bf16, name="kt")     # K'^T [d, h, s]
            vsb = res.tile([P, NS, CQ], bf16, name="vsb")   # V [t%128, tc, dv]
            wpt = res.tile([P, NHC, E], bf16, name="wpt")   # W_proj rows
            ot = res.tile([P, NHC, T], bf16, name="ot")     # O^T [dv, h, t]
            wtile = res.tile([P, 512], bf16, name="wtile")
            nc.vector.memset(wtile, 0.0)
            ones_f = res.tile([P, 1], f32, name="ones_f")
            nc.vector.memset(ones_f, 1.0)
            ones_bf = res.tile([P, 1], bf16, name="ones_bf")
            nc.vector.tensor_copy(ones_bf, ones_f)

            def qkv_rounds(t0, pa_x, pa_w, pa_ps, do_q,
                           xe_pre=None, after_k=None):
                """K, V (and optionally Q) projection rounds for one 1024-col
                t-block.  Returns the xe tiles (kept alive by pa_x pool)."""
                xe = xe_pre if xe_pre is not None else [None] * NE
                srcs = [(wk, kt, "wek"), (wv, None, "wev")] + (
                    [(wq, qt, "weq0")] if do_q else [])
                for wsrc, dst, wtag in srcs:
                    if dst is not None:
                        # K/Q round: psum[c*2+half] = [c128, t512]
                        pss = [
                            pa_ps.tile([P, 512], f32, name="psqk",
                                       tag="pa_ps", bufs=8)
                            for _ in range(8)
                        ]
                        for e in range(NE):
                            if xe[e] is None:
                                xe[e] = pa_x.tile([P, 1024], bf16,
                                                  name=f"xe{e}",
                                                  tag=f"xe{e}", bufs=1)
                                nc.sync.dma_start(
                                    xe[e],
                                    xT[e * P:(e + 1) * P, t0:t0 + 1024])
                            we = pa_w.tile([P, CQ], bf16, name="we",
                                           tag=wtag, bufs=_WE_BUFS)
                            nc.sync.dma_start(we, wsrc[e * P:(e + 1) * P, :])
                            for c in range(4):
                                for half in range(2):
                                    nc.tensor.matmul(
                                        pss[c * 2 + half],
                                        lhsT=we[:, c * P:(c + 1) * P],
                                        rhs=xe[e][:, half * 512:(half + 1) * 512],
                                        start=(e == 0), stop=(e == NE - 1),
                                    )
                        for c in range(4):
                            for half in range(2):
                                eng = nc.vector.tensor_copy if half else \
                                    nc.scalar.copy
                                eng(
                                    dst[:, c, t0 + half * 512:t0 + (half + 1) * 512],
                                    pss[c * 2 + half])
                        if after_k is not None:
                            after_k()
                            after_k = None
                    else:
                        # V round: psum[tj] = [t128, dv512]
                        psv = [
                            pa_ps.tile([P, 512], f32, name="psv",
                                       tag="pa_ps", bufs=8)
                            for _ in range(8)
                        ]
                        for e in range(NE):
                            we = pa_w.tile([P, CQ], bf16, name="we",
                                           tag=wtag, bufs=_WE_BUFS)
                            nc.sync.dma_start(we, wv[e * P:(e + 1) * P, :])
                            for tj in range(8):
                                nc.tensor.matmul(
                                    psv[tj],
                                    lhsT=xe[e][:, tj * P:(tj + 1) * P],
                                    rhs=we,
                                    start=(e == 0), stop=(e == NE - 1),
                                )
                        for tj in range(8):
                            eng = nc.vector.tensor_copy if tj % 2 else \
                                nc.scalar.copy
                            eng(vsb[:, (t0 // P) + tj, :], psv[tj])
                return xe

            # ---------- Phase A, t-block 0: K, V, Q ----------
            with (
                tc.tile_pool(name="pa_x0", bufs=1) as pa_x0,
                tc.tile_pool(name="pa_w0", bufs=1) as pa_w0,
                tc.tile_pool(name="pa_ps0", bufs=1, space="PSUM") as pa_ps0,
            ):
                # PE warm-up on memset data while the first x/W DMAs land:
                # completes the p-state ramp so real matmuls run full-speed
                wps = pa_ps0.tile([P, 512], f32, name="wups", tag="pa_ps",
                                  bufs=8)
                for _ in range(4):
                    nc.tensor.matmul(wps, lhsT=wtile[:, 0:P], rhs=wtile,
                                     start=True, stop=True)
                qkv_rounds(0, pa_x0, pa_w0, pa_ps0, do_q=True)

            # ---------- Phase A, t-block 1: K, V (Q deferred) ----------
            with tc.tile_pool(name="pa_x1", bufs=1) as pa_x1:
                with (
                    tc.tile_pool(name="pa_w1", bufs=1) as pa_w1,
                    tc.tile_pool(name="pa_ps1", bufs=1, space="PSUM") as pa_ps1,
                ):
                    wps1 = pa_ps1.tile([P, 512], f32, name="wups1",
                                       tag="pa_ps", bufs=8)
                    for _ in range(4):
                        nc.tensor.matmul(wps1, lhsT=wtile[:, 0:P],
                                         rhs=wtile, start=True, stop=True)
                    xe1 = qkv_rounds(1024, pa_x1, pa_w1, pa_ps1, do_q=False)
                    # prefetch the projection weights during attention tb0
                    nc.sync.dma_start(
                        wpt, wp.rearrange("(ho p) o -> p ho o", p=P))

                # ---------- Phase B/D: attention + deferred Q1 + out proj ---
                # PE gaps inside the ACT(exp)-bound attention iterations are
                # filled explicitly: one matmul from a "filler" generator
                # (deferred Q1 projection during t-block 0, tb0 output
                # projection during t-block 1) is emitted after each
                # attention iteration, so PE per-iter work (psa+pso+fill)
                # matches the exp cadence instead of idling.
                with (
                    tc.tile_pool(name="pb_e", bufs=1) as pb_e,
                    tc.tile_pool(name="pb_r", bufs=1) as pb_r,
                    tc.tile_pool(name="pb_m", bufs=1) as pb_m,
                    tc.tile_pool(name="pd_o", bufs=1) as pd_o,
                    tc.tile_pool(name="pq1w", bufs=1) as pq1w,
                    tc.tile_pool(name="ps_a", bufs=1, space="PSUM") as ps_a,
                    tc.tile_pool(name="ps_o", bufs=1, space="PSUM") as ps_o,
                    tc.tile_pool(name="ps_z", bufs=1, space="PSUM") as ps_z,
                    tc.tile_pool(name="ps_qd", bufs=1, space="PSUM") as ps_qd,
                ):
                    def attn_unit(h, tq, filler):
                        tr = tq * 512
                        pso = ps_o.tile([P, 512], f32, name="pso",
                                        tag="pso", bufs=_PSO_BUFS)
                        rs = pb_r.tile([P, 512], bf16, name="rs",
                                       tag="rs", bufs=3)
                        for s in range(NS):
                            psa = ps_a.tile([P, 512], f32, name="psa",
                                            tag="psa", bufs=_PSA_BUFS)
                            nc.tensor.matmul(
                                psa,
                                lhsT=kt[:, h, s * P:(s + 1) * P],
                                rhs=qt[:, h, tr:tr + 512],
                                start=True, stop=True,
                            )
                            et = pb_e.tile([P, 512], bf16, name="et",
                                           tag="et", bufs=6)
                            nc.scalar.activation(et, psa, EXP)
                            next(filler, None)
                            nc.tensor.matmul(
                                pso,
                                lhsT=vsb[:, s, h * P:(h + 1) * P],
                                rhs=et,
                                start=(s == 0), stop=(s == NS - 1),
                            )
                            if s == 0:
                                et_prev = et
                            elif s == 1:
                                nc.vector.tensor_add(rs, et_prev, et)
                            else:
                                nc.vector.tensor_add(rs, rs, et)
                        psz = ps_z.tile([1, 512], f32, name="psz",
                                        tag="ps_z", bufs=1)
                        nc.tensor.matmul(psz, lhsT=ones_bf[:, 0:1], rhs=rs,
                                         start=True, stop=True)
                        zr = pb_m.tile([1, 512], f32, name="zr",
                                       tag="zr", bufs=2)
                        nc.vector.reciprocal(zr, psz)
                        rb = pb_m.tile([P, 512], f32, name="rb",
                                       tag="rb", bufs=2)
                        nc.gpsimd.partition_broadcast(rb, zr)
                        nc.vector.tensor_mul(ot[:, h, tr:tr + 512], pso, rb)

                    def gen_q1():
                        # Deferred tb1 Q projection, one matmul per yield.
                        for c in range(NHC):
                            psq = [
                                ps_qd.tile([P, 512], f32, name="psq",
                                           tag="ps_qd", bufs=_QD_BUFS)
                                for _ in range(2)
                            ]
                            for e in range(NE):
                                weq = pq1w.tile([P, P], bf16, name="weq",
                                                tag="weq", bufs=3)
                                nc.sync.dma_start(
                                    weq,
                                    wq[e * P:(e + 1) * P, c * P:(c + 1) * P])
                                for half in range(2):
                                    nc.tensor.matmul(
                                        psq[half],
                                        lhsT=weq,
                                        rhs=xe1[e][:, half * 512:(half + 1) * 512],
                                        start=(e == 0), stop=(e == NE - 1),
                                    )
                                    yield
                            for half in range(2):
                                nc.vector.tensor_copy(
                                    qt[:, c, 1024 + half * 512:
                                       1024 + (half + 1) * 512],
                                    psq[half])

                    def gen_d(tj0, tj1, tail=False):
                        # Output projection for t rows tj0*128..tj1*128,
                        # one matmul per yield.
                        for tj in range(tj0, tj1):
                            for eo in range(4):
                                if tail and eo % 2:
                                    psd = ps_a.tile([P, 512], f32,
                                                    name="psd_t", tag="psa",
                                                    bufs=_PSA_BUFS)
                                else:
                                    psd = ps_qd.tile([P, 512], f32,
                                                     name="psd",
                                                     tag="ps_qd",
                                                     bufs=_QD_BUFS)
                                for h in range(NHC):
                                    nc.tensor.matmul(
                                        psd,
                                        lhsT=ot[:, h, tj * P:(tj + 1) * P],
                                        rhs=wpt[:, h,
                                                eo * 512:(eo + 1) * 512],
                                        start=(h == 0), stop=(h == NHC - 1),
                                    )
                                    yield
                                osb = pd_o.tile([P, 512], f32, name="osb",
                                                tag="osb", bufs=4)
                                if tail:
                                    nc.scalar.copy(osb, psd)
                                else:
                                    nc.vector.tensor_copy(osb, psd)
                                nc.sync.dma_start(
                                    out[tj * P:(tj + 1) * P,
                                        eo * 512:(eo + 1) * 512],
                                    osb)

                    def drain(gen):
                        for _ in gen:
                            pass

                    # B t-block 0 (tq 0,1): interleave the deferred Q1
                    # projection (128 matmuls over 128 attention iters).
                    q1 = gen_q1()
                    for h in range(NHC):
                        attn_unit(h, 0, q1)
                        attn_unit(h, 1, q1)
                    drain(q1)

                    # B t-block 1: interleave the tb0 output projection,
                    # then (once the tq2 units complete) the first half of
                    # the tb1 projection, at a uniform 1.5 fills per
                    # attention iter -- this balances PE per-iter work
                    # against the exp cadence across the whole block.
                    def chain2(*gens):
                        for g in gens:
                            yield from g

                    def pace(gen, per2):
                        # yields once per attention iter, pulling per2
                        # fills every 2 iters from gen
                        flip = False
                        while True:
                            for _ in range(per2 // 2 + (1 if flip and
                                           per2 % 2 else 0)):
                                next(gen, None)
                            flip = not flip
                            yield

                    d1a = chain2(gen_d(0, 8), gen_d(8, 12))
                    d_paced = pace(d1a, 3)
                    for h in range(NHC):
                        attn_unit(h, 2, d_paced)
                    for h in range(NHC):
                        attn_unit(h, 3, d_paced)
                    drain(d1a)

                    # D tail: remaining quarter of the tb1 projection
                    drain(gen_d(12, 16, tail=True))

    nc.compile()
    return nc


def _get_nc():
    global _NC_CACHE
    if _NC_CACHE is None:
        _NC_CACHE = _build_nc()
    return _NC_CACHE


def _shard_inputs(x, W_attn, W_proj, lambda_q1, lambda_k1,
                  lambda_q2, lambda_k2):
    import ml_dtypes
    x = np.asarray(x, np.float32)
    W_attn = np.asarray(W_attn, np.float32)
    W_proj = np.asarray(W_proj, np.float32)
    lam = float(np.exp(np.dot(np.asarray(lambda_q1, np.float32),
                              np.asarray(lambda_k1, np.float32)))
                - np.exp(np.dot(np.asarray(lambda_q2, np.float32),
                                np.asarray(lambda_k2, np.float32)))
                + LAMBDA_INIT)
    Cb = E // 2  # 1024: q1/k1/q2/k2 block width in W_attn
    in_maps = []
    for c in range(N_CORES):
        b, hg = divmod(c, 4)
        heads = [4 * hg + j for j in range(NHC)]
        wq_c = np.empty((E, CQ), np.float32)
        wk_c = np.empty((E, CQ), np.float32)
        wv_c = np.empty((E, CQ), np.float32)
        wp_c = np.empty((CQ, E), np.float32)
        for j, h in enumerate(heads):
            wq_c[:, j * P:j * P + HD] = W_attn[:, h * HD:(h + 1) * HD] * SCALE
            wq_c[:, j * P + HD:(j + 1) * P] = (
                W_attn[:, 2 * Cb + h * HD:2 * Cb + (h + 1) * HD]
                * (-lam * SCALE))
            wk_c[:, j * P:j * P + HD] = W_attn[:, Cb + h * HD:Cb + (h + 1) * HD]
            wk_c[:, j * P + HD:(j + 1) * P] = (
                W_attn[:, 3 * Cb + h * HD:3 * Cb + (h + 1) * HD])
            wv_c[:, j * P:(j + 1) * P] = (
                W_attn[:, 4 * Cb + h * DV:4 * Cb + (h + 1) * DV])
            wp_c[j * P:(j + 1) * P, :] = (
                W_proj[h * DV:(h + 1) * DV, :] * (1.0 - LAMBDA_INIT))
        bf = ml_dtypes.bfloat16
        in_maps.append({
            "xT": np.ascontiguousarray(x[b].T).astype(bf),
            "wq": wq_c.astype(bf), "wk": wk_c.astype(bf),
            "wv": wv_c.astype(bf),
            "wp": wp_c.astype(bf),
        })
    return in_maps


def _run(inputs, trace=False):
    from concourse.bass_utils import run_bass_kernel_spmd
    nc = _get_nc()
    in_maps = _shard_inputs(**inputs)
    res = run_bass_kernel_spmd(nc, in_maps, list(range(N_CORES)),
                               trace=trace)
    out = np.zeros((B, T, E), np.float32)
    for c in range(N_CORES):
        out[c // 4] += np.asarray(res.results[c]["out"], np.float32)
    return out, res


def kernel(x, W_attn, W_proj, lambda_q1, lambda_k1, lambda_q2, lambda_k2):
    out, _ = _run(dict(x=x, W_attn=W_attn, W_proj=W_proj,
                       lambda_q1=lambda_q1, lambda_k1=lambda_k1,
                       lambda_q2=lambda_q2, lambda_k2=lambda_k2))
    return out

